# revision 14
# baseline (speedup 1.0000x reference)
"""CGCNN (nn_CGCNNModel) on 8 trn2 NeuronCores via Bass/Tile SPMD.

V4 design (group-padded layout, activation-table-aware, fused passes):
  - edges sorted by dst; nodes split into 8 contiguous ranges of 12500; each
    core's nodes greedy-packed into G groups (<=128 whole nodes, <=KT*128 edge
    slots). All per-core node state (h, agg, AfAs/BfBs) lives in group-padded
    layout so every group's rows are a static slice.
  - gate uses sigmoid(x) = (1+tanh(x/2))/2; the 2x message scale from
    dropping the 1/2 is absorbed by BatchNorm scale-invariance (eps*4).
    Tanh and Exp share one activation table; Ln (softplus) is the only
    table-5 op and is issued once per group on a wide AP.
  - per tile: 2 PE matmuls (edge-feat, oT@AfAs) + DVE add of the gathered
    src rows (bj) into a wide SBUF preact tile; per group: one wide Tanh,
    one wide Exp, one wide Ln, one fused (1+T)*C DVE op.
  - one-hot dst matrices oT (gather) and o_t (scatter) precomputed on host
    and streamed per group (no per-tile PE transposes).
  - BN apply fused into the next layer's phase A (or the readout pooling
    pass), so h round-trips through SBUF only once per layer.
  - BfBs AllGather split into 2 chunks (chunk-major table layout) so the
    first chunk overlaps the second half of phase A.
  - direct DMAs spread across engine queues (sync/scalar/vector) to avoid
    serializing on the sync engine.
"""
import sys
import numpy as np

sys.path.insert(0, "/opt/trn_rl_repo")

import ml_dtypes

import concourse.bass as bass
import concourse.mybir as mybir
import concourse.tile as tile
from concourse import bacc
from concourse.bass_utils import run_bass_kernel_spmd
from concourse.masks import make_identity

# problem constants (hardcoded per contract)
N_NODES = 100000
N_EDGES = 800000
N_GRAPHS = 1000
F_NODE = 92
F_EDGE = 80
H = 128
N_CONV = 3
BN_EPS = 1e-5

NCORES = 8
NLOC = N_NODES // NCORES      # 12500 nodes per core
KT = 7                        # tiles per group
GSLOTS = KT * 128             # 896 edge slots per group
GB_CAP = 1024                 # graph buffer rows
OOB = 1 << 30

P = 128
f32 = mybir.dt.float32
bf16 = mybir.dt.bfloat16
i32 = mybir.dt.int32
AF = mybir.ActivationFunctionType
ALU = mybir.AluOpType

_CACHE = {}


def _bf(x):
    return np.ascontiguousarray(x).astype(ml_dtypes.bfloat16)


def pack_host(x, edge_attr, edge_index, batch, params):
    src = np.asarray(edge_index[0]).astype(np.int64)
    dst = np.asarray(edge_index[1]).astype(np.int64)
    ea = np.asarray(edge_attr, dtype=np.float32)
    batch = np.asarray(batch).astype(np.int64)
    x = np.asarray(x, dtype=np.float32)

    order = np.argsort(dst, kind="stable")
    dst_s, src_s, ea_s = dst[order], src[order], ea[order]
    deg = np.bincount(dst_s, minlength=N_NODES)
    estart = np.zeros(N_NODES + 1, dtype=np.int64)
    np.cumsum(deg, out=estart[1:])

    core_groups = []
    for c in range(NCORES):
        nlo, nhi = c * NLOC, (c + 1) * NLOC
        groups = []
        n = nlo
        while n < nhi:
            cnt = 0
            edges = 0
            while (n + cnt < nhi and cnt < 128
                   and edges + deg[n + cnt] <= GSLOTS):
                edges += int(deg[n + cnt])
                cnt += 1
            assert cnt > 0
            groups.append((n, cnt, int(estart[n]), edges))
            n += cnt
        core_groups.append(groups)
    G = max(len(g) for g in core_groups)
    ES = G * GSLOTS
    T = G * KT
    GR = G * P

    # chunk-major AllGather layout: NCHUNK chunks of groups
    NCHUNK = 4
    cb = [round(i * G / NCHUNK) for i in range(NCHUNK + 1)]  # group boundaries
    crows = [(cb[i + 1] - cb[i]) * P for i in range(NCHUNK)]
    cstart = np.zeros(NCHUNK + 1, dtype=np.int64)  # global row start per chunk
    for i in range(NCHUNK):
        cstart[i + 1] = cstart[i] + NCORES * crows[i]

    # node -> global row id in the chunk-major AllGathered table
    grow = np.zeros(N_NODES, dtype=np.int64)
    for c in range(NCORES):
        for g, (n0, cnt, e0, ecnt) in enumerate(core_groups[c]):
            ch = min(g * NCHUNK // G, NCHUNK - 1)
            while not (cb[ch] <= g < cb[ch + 1]):
                ch += 1 if g >= cb[ch + 1] else -1
            base = cstart[ch] + c * crows[ch] + (g - cb[ch]) * P
            grow[n0:n0 + cnt] = base + np.arange(cnt)

    in_maps = []
    for c in range(NCORES):
        nlo = c * NLOC
        groups = core_groups[c]
        src_slot = np.zeros(ES, dtype=np.int64)
        dloc_slot = np.full(ES, 128, dtype=np.int64)
        ea_slot = np.zeros((ES, F_EDGE), dtype=np.float32)
        xg = np.zeros((F_NODE + 1, GR), dtype=np.float32)
        for g, (n0, cnt, e0, ecnt) in enumerate(groups):
            b = g * GSLOTS
            src_slot[b:b + ecnt] = grow[src_s[e0:e0 + ecnt]]
            dloc_slot[b:b + ecnt] = dst_s[e0:e0 + ecnt] - n0
            ea_slot[b:b + ecnt] = ea_s[e0:e0 + ecnt]
            xg[:F_NODE, g * P:g * P + cnt] = x[n0:n0 + cnt].T
            xg[F_NODE, g * P:g * P + cnt] = 1.0

        eaT = np.concatenate([ea_slot.T, np.ones((1, ES), np.float32)], axis=0)
        srcT = src_slot.reshape(T, P).T.astype(np.int32).copy()

        # one-hot dst matrices per tile: oT[n, e] = (dst_local(e) == n),
        # o_t = oT.T; packed per group: [oT_t | o_t_t] blocks of 256 cols.
        dloc_t = dloc_slot.reshape(T, P)  # [T, 128] values in 0..128
        onehot = np.zeros((T, P, P + 1), dtype=np.float32)
        onehot[np.arange(T)[:, None], np.arange(P)[None, :], dloc_t] = 1.0
        o_t_all = onehot[:, :, :P]                      # [T, e, n]
        oT_all = o_t_all.transpose(0, 2, 1)             # [T, n, e]
        otg = np.empty((P, T * 2 * P), dtype=np.float32)
        for t in range(T):
            otg[:, t * 256:t * 256 + P] = oT_all[t]
            otg[:, t * 256 + P:(t + 1) * 256] = o_t_all[t]

        # pooling (group-padded rows; pad rows get zero weights)
        bl = batch[nlo:nlo + NLOC]
        g_lo = int(bl[0])
        span = int(bl[-1]) - g_lo + 1
        assert span <= 256, f"graph span {span} exceeds 2 windows"
        poolw = np.zeros((P, G * 256), dtype=np.float32)
        for g, (n0, cnt, e0, ecnt) in enumerate(groups):
            gb = batch[n0:n0 + cnt] - g_lo
            pr = np.arange(cnt)
            w = (gb // 128).astype(np.int64)
            q = (gb % 128).astype(np.int64)
            poolw[pr, g * 256 + w * 128 + q] = 1.0
        pids = np.zeros((P, 2), dtype=np.int32)
        for w in range(2):
            r = g_lo + w * 128 + np.arange(P)
            pids[:, w] = np.where(r < GB_CAP, r, OOB).astype(np.int32)

        cnts = np.bincount(batch, minlength=GB_CAP).astype(np.float32)
        invc = (1.0 / np.maximum(cnts[:GB_CAP], 1.0)).reshape(8, P).T.copy()

        lw_f = np.asarray(params["lin_f_W"], np.float32)
        lw_s = np.asarray(params["lin_s_W"], np.float32)
        lb_f = np.asarray(params["lin_f_b"], np.float32)
        lb_s = np.asarray(params["lin_s_b"], np.float32)
        # per layer: [Wi_gate|Wi_core|Wj_gate|Wj_core] -> [H, 4H] (unnegated;
        # gate goes through tanh(x/2))
        wij_all = np.concatenate(
            [np.concatenate([lw_f[l, :128], lw_s[l, :128],
                             lw_f[l, 128:256], lw_s[l, 128:256]], axis=1)
             for l in range(N_CONV)], axis=1)           # [128, 3*512]
        wfs_all = np.concatenate(
            [np.concatenate(
                [np.concatenate([lw_f[l, 256:], lw_s[l, 256:]], axis=1),
                 np.concatenate([lb_f[l], lb_s[l]])[None, :]], axis=0)
             for l in range(N_CONV)], axis=1)          # [81, 768]
        wemb = np.concatenate([np.asarray(params["emb_W"], np.float32),
                               np.asarray(params["emb_b"], np.float32)[None, :]], axis=0)

        m = {
            "xg": _bf(xg),
            "eaT": _bf(eaT),
            "srcT": srcT,
            "otg": _bf(otg),
            "wij": _bf(wij_all),
            "wfs": _bf(wfs_all),
            "wemb": _bf(wemb),
            "bng": np.asarray(params["bn_gamma"], np.float32).reshape(N_CONV, H),
            "bnb": np.asarray(params["bn_beta"], np.float32).reshape(N_CONV, H),
            "poolw": _bf(poolw),
            "pids": pids,
            "invc": invc,
            "fcw": np.asarray(params["fc_W"], np.float32),
            "fcb": np.asarray(params["fc_b"], np.float32).reshape(1, H),
            "outw": np.asarray(params["out_W"], np.float32).reshape(H)[None, :],
            "outb": np.full((P, 1), float(np.asarray(params["out_b"]).reshape(-1)[0]), np.float32),
        }
        in_maps.append(m)
    return in_maps, G


def build_program(G, reps=1):
    ES = G * GSLOTS
    T = G * KT
    GR = G * P
    NCHUNK = 4
    cb = [round(i * G / NCHUNK) for i in range(NCHUNK + 1)]
    crows = [(cb[i + 1] - cb[i]) * P for i in range(NCHUNK)]
    cstart = [0]
    for i in range(NCHUNK):
        cstart.append(cstart[-1] + NCORES * crows[i])
    nc = bacc.Bacc("TRN2", target_bir_lowering=False, debug=False, num_devices=NCORES)
    CORES = list(range(NCORES))

    xg_d = nc.dram_tensor("xg", [F_NODE + 1, GR], bf16, kind="ExternalInput")
    eaT_d = nc.dram_tensor("eaT", [F_EDGE + 1, ES], bf16, kind="ExternalInput")
    srcT_d = nc.dram_tensor("srcT", [P, T], i32, kind="ExternalInput")
    otg_d = nc.dram_tensor("otg", [P, T * 2 * P], bf16, kind="ExternalInput")
    wij_d = nc.dram_tensor("wij", [H, 4 * H * N_CONV], bf16, kind="ExternalInput")
    wfs_d = nc.dram_tensor("wfs", [F_EDGE + 1, 2 * H * N_CONV], bf16, kind="ExternalInput")
    wemb_d = nc.dram_tensor("wemb", [F_NODE + 1, H], bf16, kind="ExternalInput")
    bng_d = nc.dram_tensor("bng", [N_CONV, H], f32, kind="ExternalInput")
    bnb_d = nc.dram_tensor("bnb", [N_CONV, H], f32, kind="ExternalInput")
    poolw_d = nc.dram_tensor("poolw", [P, G * 256], bf16, kind="ExternalInput")
    pids_d = nc.dram_tensor("pids", [P, 2], i32, kind="ExternalInput")
    invc_d = nc.dram_tensor("invc", [P, GB_CAP // P], f32, kind="ExternalInput")
    fcw_d = nc.dram_tensor("fcw", [H, H], f32, kind="ExternalInput")
    fcb_d = nc.dram_tensor("fcb", [1, H], f32, kind="ExternalInput")
    outw_d = nc.dram_tensor("outw", [1, H], f32, kind="ExternalInput")
    outb_d = nc.dram_tensor("outb", [P, 1], f32, kind="ExternalInput")
    out_d = nc.dram_tensor("out", [GB_CAP, 1], f32, kind="ExternalOutput")

    h_a = nc.dram_tensor("h_a", [GR, H], f32)
    h_b = nc.dram_tensor("h_b", [GR, H], f32)
    hgrp = [h_a, h_b]
    agg_a = nc.dram_tensor("agg_a", [GR, H], f32)
    agg_b = nc.dram_tensor("agg_b", [GR, H], f32)
    agf = [agg_a, agg_b]
    afas_a = nc.dram_tensor("afas_a", [GR, 2 * H], bf16)
    afas_b = nc.dram_tensor("afas_b", [GR, 2 * H], bf16)
    afl = [afas_a, afas_b]
    bfin_a = nc.dram_tensor("bfin_a", [GR, 2 * H], bf16)
    bfin_b = nc.dram_tensor("bfin_b", [GR, 2 * H], bf16)
    bfin = [bfin_a, bfin_b]
    bfag_a = nc.dram_tensor("bfag_a", [NCORES * GR, 2 * H], bf16, addr_space="Shared")
    bfag_b = nc.dram_tensor("bfag_b", [NCORES * GR, 2 * H], bf16, addr_space="Shared")
    bfag = [bfag_a, bfag_b]
    st_in = [nc.dram_tensor(f"st_in{l}", [1, 2 * H], f32) for l in range(N_CONV)]
    st_out = [nc.dram_tensor(f"st_out{l}", [1, 2 * H], f32, addr_space="Shared")
              for l in range(N_CONV)]
    gbuf = nc.dram_tensor("gbuf", [GB_CAP, H], f32)
    gsum = nc.dram_tensor("gsum", [GB_CAP, H], f32, addr_space="Shared")

    with tile.TileContext(nc) as tc:
        with (
            tc.tile_pool(name="cst", bufs=1) as cst,
            tc.tile_pool(name="sb", bufs=3) as sb,
            tc.tile_pool(name="stream", bufs=2) as stream,
            tc.tile_pool(name="wide1", bufs=1) as wide1,
            tc.tile_pool(name="wide2", bufs=2) as wide2,
            tc.tile_pool(name="sc", bufs=2) as sc,
            tc.tile_pool(name="pst", bufs=1, space="PSUM") as pst,
            tc.tile_pool(name="psp", bufs=2, space="PSUM") as psp,
            tc.tile_pool(name="psA", bufs=2, space="PSUM") as psA_pool,
            tc.tile_pool(name="psa", bufs=1, space="PSUM") as psa,
            tc.tile_pool(name="psst", bufs=1, space="PSUM") as psst,
        ):
            id_bf = cst.tile([P, P], bf16)
            make_identity(nc, id_bf[:])
            id_f = cst.tile([P, P], f32)
            make_identity(nc, id_f[:])
            ones_col = cst.tile([P, 1], f32)
            nc.vector.memset(ones_col[:], 1.0)
            ones_row = cst.tile([1, P], f32)
            nc.vector.memset(ones_row[:], 1.0)

            srcT_t = cst.tile([P, T], i32)
            nc.sync.dma_start(out=srcT_t[:], in_=srcT_d[:])
            wij_t = cst.tile([H, 4 * H * N_CONV], bf16)
            nc.sync.dma_start(out=wij_t[:], in_=wij_d[:])
            wfs_t = cst.tile([F_EDGE + 1, 2 * H * N_CONV], bf16)
            nc.sync.dma_start(out=wfs_t[:], in_=wfs_d[:])
            wemb_t = cst.tile([F_NODE + 1, H], bf16)
            nc.sync.dma_start(out=wemb_t[:], in_=wemb_d[:])
            poolw_t = cst.tile([P, G * 256], bf16)
            nc.sync.dma_start(out=poolw_t[:], in_=poolw_d[:])
            pids_t = cst.tile([P, 2], i32)
            nc.sync.dma_start(out=pids_t[:], in_=pids_d[:])
            invc_t = cst.tile([P, GB_CAP // P], f32)
            nc.sync.dma_start(out=invc_t[:], in_=invc_d[:])
            fcw_t = cst.tile([H, H], f32)
            nc.sync.dma_start(out=fcw_t[:], in_=fcw_d[:])
            fcb_t = cst.tile([1, H], f32)
            nc.sync.dma_start(out=fcb_t[:], in_=fcb_d[:])
            outw_t = cst.tile([1, H], f32)
            nc.sync.dma_start(out=outw_t[:], in_=outw_d[:])
            outb_t = cst.tile([P, 1], f32)
            nc.sync.dma_start(out=outb_t[:], in_=outb_d[:])

            pairs = [tuple(range(gp, min(gp + 2, G))) for gp in range(0, G, 2)]

            def phase_a_tail(g, t1, l_next, h_w):
                """Shared tail of fused BN/phase-A: transpose t1, project,
                write AfAs/BfBs tables, kick chunked AllGather."""
                af_d = afl[l_next % 2]
                bf_in = bfin[l_next % 2]
                bf_ag = bfag[l_next % 2]
                wij_l = wij_t[:, l_next * 4 * H:(l_next + 1) * 4 * H]
                psT1 = pst.tile([P, H], f32, space="PSUM", tag="tr")
                nc.tensor.transpose(out=psT1[:], in_=t1[:], identity=id_f[:])
                hT = sb.tile([P, H], bf16, tag="hT")
                nc.vector.tensor_copy(out=hT[:], in_=psT1[:])
                psA = psA_pool.tile([P, 4 * H], f32, space="PSUM", tag="pA")
                nc.tensor.matmul(out=psA[:], lhsT=hT[:], rhs=wij_l,
                                 start=True, stop=True)
                afbs = sb.tile([P, 4 * H], bf16, tag="afbs")
                nc.vector.tensor_copy(out=afbs[:], in_=psA[:])
                nc.scalar.dma_start(out=af_d[g * P:(g + 1) * P, :], in_=afbs[:, :2 * H])
                nc.sync.dma_start(out=bf_in[g * P:(g + 1) * P, :], in_=afbs[:, 2 * H:])
                for ch in range(NCHUNK):
                    if g == cb[ch + 1] - 1:
                        nc.gpsimd.collective_compute(
                            "AllGather", ALU.bypass, replica_groups=[CORES],
                            ins=[bf_in[cb[ch] * P:cb[ch + 1] * P, :].opt()],
                            outs=[bf_ag[cstart[ch]:cstart[ch + 1], :].opt()])

            for _rep in range(reps):
                # ---- fused embedding + phase A(0) ----
                for g in range(G):
                    xgs = sb.tile([F_NODE + 1, P], bf16, tag="xgs")
                    nc.scalar.dma_start(out=xgs[:], in_=xg_d[:, g * P:(g + 1) * P])
                    ps = psp.tile([P, 2 * H], f32, space="PSUM", tag="pp")
                    nc.tensor.matmul(out=ps[:, :H], lhsT=xgs[:],
                                     rhs=wemb_t[:], start=True, stop=True)
                    hb = sb.tile([P, H], f32, tag="hb")
                    nc.vector.tensor_copy(out=hb[:], in_=ps[:, :H])
                    nc.sync.dma_start(out=hgrp[0][g * P:(g + 1) * P, :], in_=hb[:])
                    phase_a_tail(g, hb, 0, None)

                # ---- conv layers ----
                for l in range(N_CONV):
                    h_in = hgrp[l % 2]
                    h_out = hgrp[(l + 1) % 2]
                    wfs_l = wfs_t[:, l * 2 * H:(l + 1) * 2 * H]
                    agg_d = agf[l % 2]
                    af_d = afl[l % 2]
                    bf_ag = bfag[l % 2]
                    stat_ps = psst.tile([1, 2 * H], f32, space="PSUM", tag="stat")

                    # phase B: edge tiles, per-group wide activations
                    for pair in pairs:
                        info = {}
                        for gi, g in enumerate(pair):
                            afas_g = sb.tile([P, 2 * H], bf16, tag=f"afas{gi}")
                            nc.sync.dma_start(out=afas_g[:], in_=af_d[g * P:(g + 1) * P, :])
                            eaTg = stream.tile([F_EDGE + 1, GSLOTS], bf16, tag=f"eaTg{gi}")
                            nc.sync.dma_start(out=eaTg[:], in_=eaT_d[:, g * GSLOTS:(g + 1) * GSLOTS])
                            otg_t = stream.tile([P, KT * 2 * P], bf16, tag=f"otg{gi}")
                            nc.sync.dma_start(out=otg_t[:], in_=otg_d[:, g * KT * 2 * P:(g + 1) * KT * 2 * P])
                            pre_s = wide2.tile([P, KT * 2 * H], f32, tag=f"pre{gi}")
                            te_g = wide1.tile([P, GSLOTS], bf16, tag=f"te{gi}")
                            ex_g = wide1.tile([P, GSLOTS], f32, tag=f"ex{gi}")
                            info[g] = (otg_t, te_g, ex_g)
                            for t in range(KT):
                                k = g * KT + t
                                bj = sb.tile([P, 2 * H], bf16, tag="bj")
                                nc.gpsimd.indirect_dma_start(
                                    out=bj[:], out_offset=None, in_=bf_ag[:],
                                    in_offset=bass.IndirectOffsetOnAxis(ap=srcT_t[:, k:k + 1], axis=0))
                                pre = psp.tile([P, 2 * H], f32, space="PSUM", tag="pp")
                                nc.tensor.matmul(out=pre[:], lhsT=eaTg[:, t * 128:(t + 1) * 128],
                                                 rhs=wfs_l, start=True, stop=False)
                                nc.tensor.matmul(out=pre[:],
                                                 lhsT=otg_t[:, t * 256:t * 256 + P],
                                                 rhs=afas_g[:], start=False, stop=True)
                                nc.vector.tensor_tensor(
                                    out=pre_s[:, t * 2 * H:(t + 1) * 2 * H],
                                    in0=pre[:], in1=bj[:], op=ALU.add)
                            gate_v = pre_s[:].rearrange("p (t c) -> p t c", t=KT)[:, :, 0:H]
                            core_v = pre_s[:].rearrange("p (t c) -> p t c", t=KT)[:, :, H:2 * H]
                            te_v = te_g[:].rearrange("p (t c) -> p t c", t=KT)
                            ex_v = ex_g[:].rearrange("p (t c) -> p t c", t=KT)
                            # tanh(xg/2) and exp(xc): same activation table
                            nc.scalar.activation(out=te_v, in_=gate_v, func=AF.Tanh, scale=0.5)
                            nc.scalar.activation(out=ex_v, in_=core_v, func=AF.Exp)

                        for gi, g in enumerate(pair):
                            otg_t, te_g, ex_g = info[g]
                            # softplus core = ln(1 + exp(xc)) -- lone table-5 op
                            co_g = wide1.tile([P, GSLOTS], f32, tag=f"co{gi}")
                            nc.scalar.activation(out=co_g[:], in_=ex_g[:], func=AF.Ln, bias=1.0)
                            # msg = (1 + T) * C  == 2 * sigmoid(xg) * softplus(xc)
                            msg_g = wide2.tile([P, GSLOTS], bf16, tag=f"msg{gi}")
                            nc.vector.scalar_tensor_tensor(
                                out=msg_g[:], in0=te_g[:], scalar=1.0, in1=co_g[:],
                                op0=ALU.add, op1=ALU.mult)
                            agg_ps = psa.tile([P, H], f32, space="PSUM", tag=f"agg{gi}")
                            for t in range(KT):
                                nc.tensor.matmul(out=agg_ps[:],
                                                 lhsT=otg_t[:, t * 256 + P:(t + 1) * 256],
                                                 rhs=msg_g[:, t * P:(t + 1) * P],
                                                 start=(t == 0), stop=(t == KT - 1))
                            stat_src = sb.tile([P, 2 * H], f32, tag=f"stat_src{gi}")
                            nc.vector.tensor_copy(out=stat_src[:, :H], in_=agg_ps[:])
                            nc.vector.tensor_tensor(out=stat_src[:, H:], in0=stat_src[:, :H],
                                                    in1=agg_ps[:], op=ALU.mult)
                            nc.tensor.matmul(out=stat_ps[:], lhsT=ones_col[:], rhs=stat_src[:],
                                             start=(g == 0), stop=(g == G - 1))
                            nc.scalar.dma_start(out=agg_d[g * P:(g + 1) * P, :], in_=stat_src[:, :H])

                    # ---- BN coefficients (agg is 2x reference => 4*eps) ----
                    stat_sb = sc.tile([1, 2 * H], f32, tag="stat_sb")
                    nc.vector.tensor_copy(out=stat_sb[:], in_=stat_ps[:])
                    nc.sync.dma_start(out=st_in[l][:], in_=stat_sb[:])
                    nc.gpsimd.collective_compute(
                        "AllReduce", ALU.add, replica_groups=[CORES],
                        ins=[st_in[l][:].opt()], outs=[st_out[l][:].opt()])
                    stg = sc.tile([1, 2 * H], f32, tag="stg")
                    nc.sync.dma_start(out=stg[:], in_=st_out[l][:])
                    mean = sc.tile([1, H], f32, tag="mean")
                    nc.vector.tensor_scalar_mul(mean[:], stg[:, :H], 1.0 / N_NODES)
                    msq = sc.tile([1, H], f32, tag="msq")
                    nc.vector.tensor_scalar_mul(msq[:], stg[:, H:], 1.0 / N_NODES)
                    m2 = sc.tile([1, H], f32, tag="m2")
                    nc.vector.tensor_tensor(out=m2[:], in0=mean[:], in1=mean[:], op=ALU.mult)
                    var = sc.tile([1, H], f32, tag="var")
                    nc.vector.tensor_tensor(out=var[:], in0=msq[:], in1=m2[:], op=ALU.subtract)
                    vareps = sc.tile([1, H], f32, tag="vareps")
                    nc.vector.tensor_scalar_add(vareps[:], var[:], 4.0 * BN_EPS)
                    sd = sc.tile([1, H], f32, tag="sd")
                    nc.scalar.activation(out=sd[:], in_=vareps[:], func=AF.Sqrt)
                    rstd = sc.tile([1, H], f32, tag="rstd")
                    nc.vector.reciprocal(out=rstd[:], in_=sd[:])
                    bngl = sc.tile([1, H], f32, tag="bngl")
                    nc.sync.dma_start(out=bngl[:], in_=bng_d[l:l + 1, :])
                    bnbl = sc.tile([1, H], f32, tag="bnbl")
                    nc.sync.dma_start(out=bnbl[:], in_=bnb_d[l:l + 1, :])
                    gco = sc.tile([1, H], f32, tag="gco")
                    nc.vector.tensor_tensor(out=gco[:], in0=rstd[:], in1=bngl[:], op=ALU.mult)
                    mg = sc.tile([1, H], f32, tag="mg")
                    nc.vector.tensor_tensor(out=mg[:], in0=mean[:], in1=gco[:], op=ALU.mult)
                    bco = sc.tile([1, H], f32, tag="bco")
                    nc.vector.tensor_tensor(out=bco[:], in0=bnbl[:], in1=mg[:], op=ALU.subtract)
                    psGB = pst.tile([P, H], f32, space="PSUM", tag="tr")
                    nc.tensor.matmul(out=psGB[:], lhsT=ones_row[:], rhs=gco[:], start=True, stop=True)
                    GBt = sc.tile([P, H], f32, tag="GBt")
                    nc.vector.tensor_copy(out=GBt[:], in_=psGB[:])
                    psBB = pst.tile([P, H], f32, space="PSUM", tag="tr")
                    nc.tensor.matmul(out=psBB[:], lhsT=ones_row[:], rhs=bco[:], start=True, stop=True)
                    BBt = sc.tile([P, H], f32, tag="BBt")
                    nc.vector.tensor_copy(out=BBt[:], in_=psBB[:])

                    # ---- fused BN apply + residual + next phase A / pooling ----
                    if l < N_CONV - 1:
                        for g in range(G):
                            ab = sb.tile([P, H], f32, tag="ab")
                            nc.gpsimd.dma_start(out=ab[:], in_=agg_d[g * P:(g + 1) * P, :])
                            ho = sb.tile([P, H], f32, tag="ho")
                            nc.scalar.dma_start(out=ho[:], in_=h_in[g * P:(g + 1) * P, :])
                            t1 = sb.tile([P, H], f32, tag="t1")
                            nc.vector.tensor_tensor(out=t1[:], in0=ab[:], in1=GBt[:], op=ALU.mult)
                            nc.vector.tensor_tensor(out=t1[:], in0=t1[:], in1=BBt[:], op=ALU.add)
                            nc.vector.tensor_tensor(out=t1[:], in0=t1[:], in1=ho[:], op=ALU.add)
                            nc.sync.dma_start(out=h_out[g * P:(g + 1) * P, :], in_=t1[:])
                            phase_a_tail(g, t1, l + 1, None)
                    else:
                        psW0 = psa.tile([P, H], f32, space="PSUM", tag="agg0")
                        psW1 = psa.tile([P, H], f32, space="PSUM", tag="agg1")
                        psW = [psW0, psW1]
                        for g in range(G):
                            ab = sb.tile([P, H], f32, tag="ab")
                            nc.gpsimd.dma_start(out=ab[:], in_=agg_d[g * P:(g + 1) * P, :])
                            ho = sb.tile([P, H], f32, tag="ho")
                            nc.scalar.dma_start(out=ho[:], in_=h_in[g * P:(g + 1) * P, :])
                            t1 = sb.tile([P, H], f32, tag="t1")
                            nc.vector.tensor_tensor(out=t1[:], in0=ab[:], in1=GBt[:], op=ALU.mult)
                            nc.vector.tensor_tensor(out=t1[:], in0=t1[:], in1=BBt[:], op=ALU.add)
                            hb3 = sb.tile([P, H], bf16, tag="hb3")
                            nc.vector.scalar_tensor_tensor(
                                out=hb3[:], in0=t1[:], scalar=0.0, in1=ho[:],
                                op0=ALU.add, op1=ALU.add)
                            for w in range(2):
                                nc.tensor.matmul(
                                    out=psW[w][:],
                                    lhsT=poolw_t[:, g * 256 + w * 128:g * 256 + (w + 1) * 128],
                                    rhs=hb3[:], start=(g == 0), stop=(g == G - 1))

                # ---- readout ----
                zt = sb.tile([P, H], f32, tag="zt")
                nc.vector.memset(zt[:], 0.0)
                for i in range(GB_CAP // P):
                    nc.sync.dma_start(out=gbuf[i * P:(i + 1) * P, :], in_=zt[:])
                for w in range(2):
                    ws = sb.tile([P, H], f32, tag="ws")
                    nc.vector.tensor_copy(out=ws[:], in_=psW[w][:])
                    nc.gpsimd.indirect_dma_start(
                        out=gbuf[:],
                        out_offset=bass.IndirectOffsetOnAxis(ap=pids_t[:, w:w + 1], axis=0),
                        in_=ws[:], in_offset=None,
                        bounds_check=GB_CAP - 1, oob_is_err=False)
                nc.gpsimd.collective_compute(
                    "AllReduce", ALU.add, replica_groups=[CORES],
                    ins=[gbuf[:].opt()], outs=[gsum[:].opt()])

                psOW = pst.tile([P, H], f32, space="PSUM", tag="tr")
                nc.tensor.matmul(out=psOW[:], lhsT=ones_row[:], rhs=outw_t[:], start=True, stop=True)
                owb = sc.tile([P, H], f32, tag="owb")
                nc.vector.tensor_copy(out=owb[:], in_=psOW[:])

                # softplus(fc) with Exp loop then Ln loop (one table swap)
                exw = cst.tile([P, GB_CAP], f32)
                spw = cst.tile([P, GB_CAP], f32)
                for gb in range(GB_CAP // P):
                    gl = sb.tile([P, H], f32, tag="gl")
                    nc.gpsimd.dma_start(out=gl[:], in_=gsum[gb * P:(gb + 1) * P, :])
                    gm = sb.tile([P, H], f32, tag="gm")
                    nc.vector.tensor_scalar(out=gm[:], in0=gl[:], scalar1=invc_t[:, gb:gb + 1],
                                            scalar2=None, op0=ALU.mult)
                    psT2 = pst.tile([P, H], f32, space="PSUM", tag="tr")
                    nc.tensor.transpose(out=psT2[:], in_=gm[:], identity=id_f[:])
                    gT2 = sb.tile([P, H], f32, tag="gT2")
                    nc.vector.tensor_copy(out=gT2[:], in_=psT2[:])
                    psF = psp.tile([P, 2 * H], f32, space="PSUM", tag="pp")
                    nc.tensor.matmul(out=psF[:, :H], lhsT=gT2[:], rhs=fcw_t[:], start=True, stop=False)
                    nc.tensor.matmul(out=psF[:, :H], lhsT=ones_row[:], rhs=fcb_t[:], start=False, stop=True)
                    nc.scalar.activation(out=exw[:, gb * P:(gb + 1) * P], in_=psF[:, :H], func=AF.Exp)
                for gb in range(GB_CAP // P):
                    nc.scalar.activation(out=spw[:, gb * P:(gb + 1) * P],
                                         in_=exw[:, gb * P:(gb + 1) * P], func=AF.Ln, bias=1.0)
                    mu = sb.tile([P, H], f32, tag="mu")
                    nc.vector.tensor_tensor(out=mu[:], in0=spw[:, gb * P:(gb + 1) * P],
                                            in1=owb[:], op=ALU.mult)
                    red = sb.tile([P, 1], f32, tag="red")
                    nc.vector.tensor_reduce(out=red[:], in_=mu[:], axis=mybir.AxisListType.X, op=ALU.add)
                    redb = sb.tile([P, 1], f32, tag="redb")
                    nc.vector.tensor_scalar(out=redb[:], in0=red[:], scalar1=outb_t[:, :1],
                                            scalar2=None, op0=ALU.add)
                    nc.sync.dma_start(out=out_d[gb * P:(gb + 1) * P, :], in_=redb[:])

    nc.compile()
    return nc


def get_program(G, reps=1):
    key = (G, reps)
    if key not in _CACHE:
        _CACHE[key] = build_program(G, reps)
    return _CACHE[key]


def kernel(x, edge_attr, emb_W, emb_b, lin_f_W, lin_f_b, lin_s_W, lin_s_b,
           bn_gamma, bn_beta, fc_W, fc_b, out_W, out_b, edge_index, batch):
    params = dict(emb_W=emb_W, emb_b=emb_b, lin_f_W=lin_f_W, lin_f_b=lin_f_b,
                  lin_s_W=lin_s_W, lin_s_b=lin_s_b, bn_gamma=bn_gamma,
                  bn_beta=bn_beta, fc_W=fc_W, fc_b=fc_b, out_W=out_W, out_b=out_b)
    in_maps, G = pack_host(x, edge_attr, edge_index, batch, params)
    nc = get_program(G)
    res = run_bass_kernel_spmd(nc, in_maps, list(range(NCORES)))
    out = res.results[0]["out"]
    return np.asarray(out, dtype=np.float32).reshape(GB_CAP)[:N_GRAPHS]
